# revision 38
# baseline (speedup 1.0000x reference)
"""Multi-head attention (B=4, S=2048, D=1024, H=16, causal) on 8 trn2 cores.

Sharding: core c -> (batch b = c//2, head-half g = c%2, heads g*8..g*8+8).
Each core computes QKV projections for its 8 heads, causal attention, and a
partial dense projection (its 512 input dims). Host sums core pairs + bias.

On-chip layout (per core), all matmuls bf16 with fp32 PSUM accumulate:
  qT/kT/vT  [d, s]   transposed on host, streamed per 512-col s-chunk
  QT/KT     [m, s]   m = 8*64 head dims on partitions (4 tiles of 128);
                     bias fused into the PSUM eviction (tensor_scalar_add)
  VA        [s, 8*65] per 128-row s-tile; col 64 of each 65-group = ones
                     (gives softmax denominators via the P@V matmul)
  attention: per 512-wide q-chunk c (4 chunks) and head pair hp:
             ST12 psum [128, 2, 512] = K_h @ Q_h^T for both heads
             (row-packed via tile_position, adjacent banks); ONE exp
             ACTIVATE over both heads per k-tile (scale=1/8, no
             max-subtraction: logits are O(1)); causal = skip
             above-diagonal k-tiles + triangular mask mul on diag block
  O psum    [65, 512] per head: rows 0:64 = unnormalized O^T, row 64 =
            softmax sums
  evict     O -> SBUF copy (frees PSUM banks fast); sums rows gathered via
            DMA into [32,32] so the reciprocal runs on 32 DVE lanes, then
            DRAM-broadcast back to [128, 512] (both heads) for the
            normalize muls
  dense     out[s, 1024] partial = O^T.T @ dwT, evicted fp32, emitted per
            q-chunk right after its attention so it fills ACT-bound PE gaps

The 512-wide chunking interleaves attention's ACT-bound stretches with
projection/dense PE work across the whole kernel, keeping the PE array
busy (HAM stays at full clock).
"""

import sys

sys.path.insert(0, "/opt/trn_rl_repo")

import numpy as np
import ml_dtypes

import concourse.bass as bass
import concourse.mybir as mybir
import concourse.tile as tile
from concourse.bass_utils import run_bass_kernel_spmd

BF16 = mybir.dt.bfloat16
F32 = mybir.dt.float32
bf16 = ml_dtypes.bfloat16

B, S, D, H, DEPTH = 4, 2048, 1024, 16, 64
NCORES = 8
HPC = H // 2  # 8 heads per core
M = HPC * DEPTH  # 512 head dims per core
CH = 512  # attention s_q chunk width
NCH = S // CH  # 4
NKT = S // 128  # 16 k tiles
EXPF = mybir.ActivationFunctionType.Exp

_CACHE = {}


def _proj_chunk(tc, sc, xch, pjps, tensors, wdma=None):
    """Emit projection work for one 512-wide s-chunk. One batched DMA per
    input tensor (sync-engine DMA dispatch is ~0.6us each — batching is
    what keeps the dispatch stream off the critical path). For chunk 0,
    `wdma` interleaves the weight loads between them."""
    nc = tc.nc
    (qTr, kTr, vTr, wq_sb, wk_sb, wv_sb, bqT_sb, bkT_sb, bvb_sb, QT, KT, VA) = tensors
    ssl = bass.ts(sc, 512)
    q_ch = xch.tile([128, 8, 512], BF16, tag="q_ch", name=f"q_ch{sc}")
    k_ch = xch.tile([128, 8, 512], BF16, tag="k_ch", name=f"k_ch{sc}")
    v_ch = xch.tile([128, 8, 512], BF16, tag="v_ch", name=f"v_ch{sc}")
    if wdma is not None:
        wdma(0)
    nc.sync.dma_start(out=q_ch, in_=qTr[:, :, ssl])
    if wdma is not None:
        wdma(1)
    nc.sync.dma_start(out=k_ch, in_=kTr[:, :, ssl])
    if wdma is not None:
        wdma(2)
    nc.sync.dma_start(out=v_ch, in_=vTr[:, :, ssl])
    for mt in range(4):
        msl = bass.ts(mt, 128)
        ps_q = pjps.tile([128, 512], F32, tag="pj", name=f"psq{sc}_{mt}")
        for t in range(8):
            nc.tensor.matmul(
                ps_q, wq_sb[:, t, msl], q_ch[:, t, :], start=(t == 0), stop=(t == 7)
            )
        nc.vector.tensor_scalar_add(QT[mt][:, ssl], ps_q, bqT_sb[:, mt : mt + 1])
        ps_k = pjps.tile([128, 512], F32, tag="pj", name=f"psk{sc}_{mt}")
        for t in range(8):
            nc.tensor.matmul(
                ps_k, wk_sb[:, t, msl], k_ch[:, t, :], start=(t == 0), stop=(t == 7)
            )
        nc.vector.tensor_scalar_add(KT[mt][:, ssl], ps_k, bkT_sb[:, mt : mt + 1])
    for sti in range(4):  # V: [s part, m free]
        st = sc * 4 + sti
        ps_v = pjps.tile([128, 512], F32, tag="pj", name=f"psv{st}")
        for t in range(8):
            nc.tensor.matmul(
                ps_v, v_ch[:, t, bass.ts(sti, 128)], wv_sb[:, t, :],
                start=(t == 0), stop=(t == 7),
            )
        nc.vector.tensor_add(
            VA[st].rearrange("p (h c) -> p h c", c=65)[:, :, 0:64],
            ps_v.rearrange("p (h c) -> p h c", c=64),
            bvb_sb.rearrange("p (h c) -> p h c", c=64),
        )


def _attn_chunk(tc, c, hp, pools, tensors):
    """Causal attention for 512-wide s_q chunk c, head pair hp."""
    nc = tc.nc
    stps, ops, epool, spool, gpool, rpool, tpool = pools
    (QT, KT, VA, OT, mask_sb, ones_t) = tensors
    csl = bass.ts(c, CH)
    O1 = ops.tile([65, CH], F32, tag="O1", name=f"O1_{c}_{hp}")
    O2 = ops.tile([65, CH], F32, tag="O2", name=f"O2_{c}_{hp}")
    nkt = 4 * (c + 1)
    for kt in range(nkt):
        j = kt - 4 * c
        qoff = 128 * j if j > 0 else 0
        ksl = bass.ts(kt, 128)
        ST12 = stps.tile([128, 2, CH], F32, tag="ST", name=f"ST_{c}_{hp}_{kt}")
        for idx in (0, 1):
            bp = idx * 64
            nc.tensor.matmul(
                ST12[:, idx, qoff:CH],
                KT[hp][bp : bp + 64, ksl],
                QT[hp][bp : bp + 64, bass.ds(c * CH + qoff, CH - qoff)],
                start=True, stop=True,
                tile_position=(bp, 0),
            )
        E12 = epool.tile([128, 2, CH], BF16, tag="E", name=f"E_{c}_{hp}_{kt}")
        nc.scalar.activation(
            E12[:, :, qoff:CH], ST12[:, :, qoff:CH], EXPF, scale=0.125
        )
        if j >= 0:
            dsl = bass.ds(qoff, 128)
            nc.vector.tensor_mul(E12[:, 0, dsl], E12[:, 0, dsl], mask_sb)
            nc.vector.tensor_mul(E12[:, 1, dsl], E12[:, 1, dsl], mask_sb)
        for idx, Ox in ((0, O1), (1, O2)):
            h = 2 * hp + idx
            nc.tensor.matmul(
                Ox[:, qoff:CH],
                VA[kt][:, h * 65 : (h + 1) * 65],
                E12[:, idx, qoff:CH],
                start=(kt == 0), stop=(kt == nkt - 1),
            )
    # evict: copy O to SBUF (frees PSUM banks), wide-lane reciprocal of the
    # softmax sums via a [32,32] reshape, then broadcast across partitions
    # with a K=1 bf16 outer-product matmul (ones[1,64].T @ row[1,512] ->
    # [64,512] PSUM, ~0.2us, into the just-freed O slots) instead of a
    # DRAM round-trip, and normalize.
    hslot = c * 4 + hp
    Ss = []
    g = gpool.tile([32, 32], F32, tag="g", name=f"g{hslot}")
    for idx, Ox in ((0, O1), (1, O2)):
        S = spool.tile([65, CH], F32, tag="S", name=f"S{hslot}_{idx}")
        nc.vector.tensor_copy(S, Ox)
        nc.gpsimd.dma_start(out=g[16 * idx : 16 * idx + 16, :], in_=S[64:65, :])
        Ss.append(S)
    gr = gpool.tile([32, 32], F32, tag="gr", name=f"gr{hslot}")
    nc.vector.reciprocal(gr, g)
    grb = gpool.tile([32, 32], BF16, tag="grb", name=f"grb{hslot}")
    nc.vector.tensor_copy(grb, gr)
    bcs = []
    for idx in (0, 1):
        r = rpool.tile([1, CH], BF16, tag="r", name=f"r{hslot}_{idx}")
        nc.gpsimd.dma_start(out=r, in_=grb[16 * idx : 16 * idx + 16, :])
        bc = ops.tile([65, CH], F32, tag=("O1", "O2")[idx], name=f"bc{idx}_{hslot}")
        nc.tensor.matmul(
            bc[0:64, :], ones_t[0:1, :], r, start=True, stop=True,
        )
        bcs.append(bc)
    nc.vector.tensor_mul(OT[hp][0:64, csl], Ss[0][0:64, :], bcs[0][0:64, :])
    tmp = tpool.tile([64, CH], BF16, tag="tmp", name=f"tmp{hslot}")
    nc.vector.tensor_mul(tmp, Ss[1][0:64, :], bcs[1][0:64, :])
    nc.gpsimd.dma_start(out=OT[hp][64:128, csl], in_=tmp)


def _body(tc):
    nc = tc.nc
    dram = {t.name: t for t in _CACHE["dram"]}
    out = dram["out"]
    rb = _CACHE["rb"]

    # ---- persistent tiles (one bufs=1 pool, distinct tags -> own slots) ----
    import contextlib
    _pc = contextlib.ExitStack()
    persist = _pc.enter_context(tc.tile_pool(name="persist", bufs=1))

    def P(shape, dt, name):
        return persist.tile(shape, dt, tag=name, name=name)

    wq_sb = P([128, 8, M], BF16, "wq_sb")
    wk_sb = P([128, 8, M], BF16, "wk_sb")
    wv_sb = P([128, 8, M], BF16, "wv_sb")
    dw_sb = P([128, 4, D], BF16, "dw_sb")
    bqT_sb = P([128, 4], F32, "bqT_sb")
    bkT_sb = P([128, 4], F32, "bkT_sb")
    bvb_sb = P([128, M], F32, "bvb_sb")
    mask_sb = P([128, 128], BF16, "mask_sb")

    wqr = dram["wqT"][:, :].rearrange("(t p) m -> p t m", p=128)
    wkr = dram["wkT"][:, :].rearrange("(t p) m -> p t m", p=128)
    wvr = dram["wvT"][:, :].rearrange("(t p) m -> p t m", p=128)

    def wdma(i):
        # bulk weight loads, interleaved with chunk-0 input DMAs
        if i == 0:
            nc.sync.dma_start(out=wq_sb, in_=wqr)
        elif i == 1:
            nc.sync.dma_start(out=wk_sb, in_=wkr)
        else:
            nc.sync.dma_start(out=wv_sb, in_=wvr)

    QT = {mt: P([128, S], BF16, f"QT{mt}") for mt in range(4)}
    KT = {mt: P([128, S], BF16, f"KT{mt}") for mt in range(4)}
    VA = {st: P([128, HPC * 65], BF16, f"VA{st}") for st in range(NKT)}
    OT = {hp: P([128, S], BF16, f"OT{hp}") for hp in range(4)}
    ones_t = P([1, 64], BF16, "ones_t")
    nc.vector.memset(ones_t, 1.0)
    for st in range(NKT):
        nc.vector.memset(VA[st], 1.0)

    qTr = dram["qT"][:, :].rearrange("(t p) s -> p t s", p=128)
    kTr = dram["kT"][:, :].rearrange("(t p) s -> p t s", p=128)
    vTr = dram["vT"][:, :].rearrange("(t p) s -> p t s", p=128)
    ptens = (qTr, kTr, vTr, wq_sb, wk_sb, wv_sb, bqT_sb, bkT_sb, bvb_sb, QT, KT, VA)
    atens = (QT, KT, VA, OT, mask_sb, ones_t)

    with (
        tc.tile_pool(name="xch", bufs=2) as xch,
        tc.tile_pool(name="stps", bufs=2, space="PSUM") as stps,
        tc.tile_pool(name="ops", bufs=1, space="PSUM") as ops,
        tc.tile_pool(name="pjps", bufs=2, space="PSUM") as pjps,
        tc.tile_pool(name="epool", bufs=3) as epool,
        tc.tile_pool(name="spool", bufs=3) as spool,
        tc.tile_pool(name="gpool", bufs=2) as gpool,
        tc.tile_pool(name="rpool", bufs=2) as rpool,
        tc.tile_pool(name="tpool", bufs=2) as tpool,
        tc.tile_pool(name="osb", bufs=3) as osb,
    ):
        apools = (stps, ops, epool, spool, gpool, rpool, tpool)

        def dense_st(st):
            ssl = bass.ts(st, 128)
            o_sb = osb.tile([128, D], BF16, tag="o_sb", name=f"o_sb{st}")
            for nh in range(2):
                ps = pjps.tile([128, 512], F32, tag="pj", name=f"dn{st}_{nh}")
                for mt in range(4):
                    nc.tensor.matmul(
                        ps, OT[mt][:, ssl], dw_sb[:, mt, bass.ts(nh, 512)],
                        start=(mt == 0), stop=(mt == 3),
                    )
                nc.vector.tensor_copy(o_sb[:, bass.ts(nh, 512)], ps)
            nc.sync.dma_start(out=out[bass.ts(st, 128), :], in_=o_sb)

        # emission order = Tile priority. Attention chunk c only needs
        # QT cols [512c:512c+512) and KT/VA cols [0:512c+512) — i.e. proj
        # chunks 0..c — so each attention chunk is emitted right after the
        # proj chunk it unblocks. Later proj chunks and per-chunk dense
        # matmuls then fill attention's ACT-bound PE gaps.
        _proj_chunk(tc, 0, xch, pjps, ptens, wdma=wdma)
        nc.sync.dma_start(out=bqT_sb, in_=dram["bqT"][:, :])
        nc.sync.dma_start(out=bkT_sb, in_=dram["bkT"][:, :])
        nc.sync.dma_start(out=mask_sb, in_=dram["mask"][:, :])
        nc.sync.dma_start(out=bvb_sb, in_=dram["bvb"][:, :])
        for c in range(NCH):
            for hp in range(4):
                _attn_chunk(tc, c, hp, apools, atens)
            if c == 0:
                nc.sync.dma_start(
                    out=dw_sb,
                    in_=dram["dwT"][:, :].rearrange("(t p) d -> p t d", p=128),
                )
            if c + 1 < NCH:
                _proj_chunk(tc, c + 1, xch, pjps, ptens)
            if c > 0:
                for st in range(4 * (c - 1), 4 * c):
                    dense_st(st)
        for st in range(4 * (NCH - 1), 4 * NCH):
            dense_st(st)
    _pc.close()


def _legalize_dma_waits(nc):
    """Walrus accepts only one sync wait per instruction (EventSemaphore: 2,
    Drain: special-cased). Spill extra waits onto preceding InstEventSemaphore
    ops on the same engine sequencer."""
    for f in nc.m.functions:
        for blk in f.blocks:
            new_insts = []
            for inst in blk.instructions:
                si = getattr(inst, "sync_info", None)
                exempt = isinstance(inst, mybir.InstEventSemaphore)
                if not exempt and si is not None and len(si.on_wait) > 1:
                    waits = list(si.on_wait)
                    extra, keep = waits[:-1], waits[-1:]
                    while extra:
                        chunk, extra = extra[:2], extra[2:]
                        new_insts.append(
                            mybir.InstEventSemaphore(
                                name=nc.get_next_instruction_name(),
                                engine=inst.engine,
                                ins=[],
                                outs=[],
                                sync_info=mybir.SyncInfo(on_wait=chunk, on_update=[]),
                            )
                        )
                    inst.sync_info = mybir.SyncInfo(
                        on_wait=keep, on_update=list(si.on_update)
                    )
                new_insts.append(inst)
            blk.instructions[:] = new_insts


def _build():
    nc = bass.Bass()
    dram = [
        nc.declare_dram_parameter("qT", [D, S], BF16, isOutput=False),
        nc.declare_dram_parameter("kT", [D, S], BF16, isOutput=False),
        nc.declare_dram_parameter("vT", [D, S], BF16, isOutput=False),
        nc.declare_dram_parameter("wqT", [D, M], BF16, isOutput=False),
        nc.declare_dram_parameter("wkT", [D, M], BF16, isOutput=False),
        nc.declare_dram_parameter("wvT", [D, M], BF16, isOutput=False),
        nc.declare_dram_parameter("dwT", [M, D], BF16, isOutput=False),
        nc.declare_dram_parameter("bqT", [128, 4], F32, isOutput=False),
        nc.declare_dram_parameter("bkT", [128, 4], F32, isOutput=False),
        nc.declare_dram_parameter("bvb", [128, M], F32, isOutput=False),
        nc.declare_dram_parameter("mask", [128, 128], BF16, isOutput=False),
        nc.declare_dram_parameter("out", [S, D], BF16, isOutput=True),
    ]
    _CACHE["dram"] = dram
    _CACHE["rb"] = nc.dram_tensor("rb", [16, 2 * CH], F32)
    with tile.TileContext(nc) as tc:
        _body(tc)
    _legalize_dma_waits(nc)
    return nc


def _get_nc():
    if "nc" not in _CACHE:
        _CACHE["nc"] = _build()
    return _CACHE["nc"]


def _make_in_maps(q, k, v, wq_w, wq_b, wk_w, wk_b, wv_w, wv_b, dense_w, dense_b):
    q, k, v = (np.asarray(x, np.float32) for x in (q, k, v))
    mask = np.triu(np.ones((128, 128), np.float32)).astype(bf16)
    in_maps = []
    for core in range(NCORES):
        b, g = divmod(core, 2)
        hs = slice(g * M, (g + 1) * M)
        in_maps.append(
            {
                "qT": q[b].T.astype(bf16),
                "kT": k[b].T.astype(bf16),
                "vT": v[b].T.astype(bf16),
                "wqT": np.asarray(wq_w)[hs].T.astype(bf16),
                "wkT": np.asarray(wk_w)[hs].T.astype(bf16),
                "wvT": np.asarray(wv_w)[hs].T.astype(bf16),
                "dwT": np.asarray(dense_w)[:, hs].T.astype(bf16),
                "bqT": np.ascontiguousarray(
                    np.asarray(wq_b)[hs].reshape(4, 128).T
                ).astype(np.float32),
                "bkT": np.ascontiguousarray(
                    np.asarray(wk_b)[hs].reshape(4, 128).T
                ).astype(np.float32),
                "bvb": np.ascontiguousarray(
                    np.broadcast_to(np.asarray(wv_b)[hs], (128, M))
                ).astype(np.float32),
                "mask": mask,
            }
        )
    return in_maps


def kernel(q, k, v, wq_w, wq_b, wk_w, wk_b, wv_w, wv_b, dense_w, dense_b):
    nc = _get_nc()
    in_maps = _make_in_maps(
        q, k, v, wq_w, wq_b, wk_w, wk_b, wv_w, wv_b, dense_w, dense_b
    )
    res = run_bass_kernel_spmd(nc, in_maps, list(range(NCORES)))
    _CACHE["last_res"] = res
    outs = [r["out"] for r in res.results]
    final = np.empty((B, S, D), np.float32)
    db = np.asarray(dense_b, np.float32)
    for b in range(B):
        final[b] = (
            outs[2 * b].astype(np.float32)
            + outs[2 * b + 1].astype(np.float32)
            + db[None, :]
        )
    return final


# revision 42
# speedup vs baseline: 1.1677x; 1.1677x over previous
"""Multi-head attention (B=4, S=2048, D=1024, H=16, causal) on 8 trn2 cores.

Sharding: core c -> (batch b = c//2, head-half g = c%2, heads g*8..g*8+8).
Each core computes QKV projections for its 8 heads, causal attention, and a
partial dense projection (its 512 input dims). Host sums core pairs + bias.

On-chip layout (per core), all matmuls bf16 with fp32 PSUM accumulate:
  qT/kT/vT  [d, s]   transposed on host, streamed per 512-col s-chunk
  QT/KT     [m, s]   m = 8*64 head dims on partitions (4 tiles of 128);
                     bias fused into the PSUM eviction (tensor_scalar_add)
  VA        [s, 8*65] per 128-row s-tile; col 64 of each 65-group = ones
                     (gives softmax denominators via the P@V matmul)
  attention: per 512-wide q-chunk c (4 chunks) and head pair hp:
             ST12 psum [128, 2, 512] = K_h @ Q_h^T for both heads
             (row-packed via tile_position, adjacent banks); ONE exp
             ACTIVATE over both heads per k-tile (scale=1/8, no
             max-subtraction: logits are O(1)); causal = skip
             above-diagonal k-tiles + triangular mask mul on diag block
  O psum    [65, 512] per head: rows 0:64 = unnormalized O^T, row 64 =
            softmax sums
  evict     O -> SBUF copy (frees PSUM banks fast); sums rows gathered via
            DMA into [32,32] so the reciprocal runs on 32 DVE lanes, then
            DRAM-broadcast back to [128, 512] (both heads) for the
            normalize muls
  dense     out[s, 1024] partial = O^T.T @ dwT, evicted fp32, emitted per
            q-chunk right after its attention so it fills ACT-bound PE gaps

The 512-wide chunking interleaves attention's ACT-bound stretches with
projection/dense PE work across the whole kernel, keeping the PE array
busy (HAM stays at full clock).
"""

import sys

sys.path.insert(0, "/opt/trn_rl_repo")

import numpy as np
import ml_dtypes

import concourse.bass as bass
import concourse.mybir as mybir
import concourse.tile as tile
from concourse.bass_utils import run_bass_kernel_spmd

BF16 = mybir.dt.bfloat16
F32 = mybir.dt.float32
bf16 = ml_dtypes.bfloat16

B, S, D, H, DEPTH = 4, 2048, 1024, 16, 64
NCORES = 8
HPC = H // 2  # 8 heads per core
M = HPC * DEPTH  # 512 head dims per core
CH = 512  # attention s_q chunk width
NCH = S // CH  # 4
NKT = S // 128  # 16 k tiles
EXPF = mybir.ActivationFunctionType.Exp

_CACHE = {}


def _proj_chunk(tc, sc, xch, pjps, tensors, wdma=None):
    """Emit projection work for one 512-wide s-chunk. One batched DMA per
    input tensor (sync-engine DMA dispatch is ~0.6us each — batching is
    what keeps the dispatch stream off the critical path). For chunk 0,
    `wdma` interleaves the weight loads between them."""
    nc = tc.nc
    (qTr, kTr, vTr, wq_sb, wk_sb, wv_sb, bqT_sb, bkT_sb, bvb_sb, QT, KT, VA) = tensors
    ssl = bass.ts(sc, 512)
    q_ch = xch.tile([128, 8, 512], BF16, tag="q_ch", name=f"q_ch{sc}")
    k_ch = xch.tile([128, 8, 512], BF16, tag="k_ch", name=f"k_ch{sc}")
    v_ch = xch.tile([128, 8, 512], BF16, tag="v_ch", name=f"v_ch{sc}")
    if wdma is not None:
        wdma(0)
    nc.sync.dma_start(out=q_ch, in_=qTr[:, :, ssl])
    if wdma is not None:
        wdma(1)
    nc.sync.dma_start(out=k_ch, in_=kTr[:, :, ssl])
    if wdma is not None:
        wdma(2)
    nc.sync.dma_start(out=v_ch, in_=vTr[:, :, ssl])
    for mt in range(4):
        msl = bass.ts(mt, 128)
        ps_q = pjps.tile([128, 512], F32, tag="pj", name=f"psq{sc}_{mt}")
        for t in range(8):
            nc.tensor.matmul(
                ps_q, wq_sb[:, t, msl], q_ch[:, t, :], start=(t == 0), stop=(t == 7)
            )
        nc.vector.tensor_scalar_add(QT[mt][:, ssl], ps_q, bqT_sb[:, mt : mt + 1])
        ps_k = pjps.tile([128, 512], F32, tag="pj", name=f"psk{sc}_{mt}")
        for t in range(8):
            nc.tensor.matmul(
                ps_k, wk_sb[:, t, msl], k_ch[:, t, :], start=(t == 0), stop=(t == 7)
            )
        nc.vector.tensor_scalar_add(KT[mt][:, ssl], ps_k, bkT_sb[:, mt : mt + 1])
    for sti in range(4):  # V: [s part, m free]
        st = sc * 4 + sti
        ps_v = pjps.tile([128, 512], F32, tag="pj", name=f"psv{st}")
        for t in range(8):
            nc.tensor.matmul(
                ps_v, v_ch[:, t, bass.ts(sti, 128)], wv_sb[:, t, :],
                start=(t == 0), stop=(t == 7),
            )
        nc.vector.tensor_add(
            VA[st].rearrange("p (h c) -> p h c", c=65)[:, :, 0:64],
            ps_v.rearrange("p (h c) -> p h c", c=64),
            bvb_sb.rearrange("p (h c) -> p h c", c=64),
        )


def _attn_chunk(tc, c, hp, pools, tensors):
    """Causal attention for 512-wide s_q chunk c, head pair hp."""
    nc = tc.nc
    stps, ops, epool, spool, gpool, bcpool, tpool = pools
    (QT, KT, VA, OT, mask_sb, rb) = tensors
    csl = bass.ts(c, CH)
    O1 = ops.tile([65, CH], F32, tag="O1", name=f"O1_{c}_{hp}")
    O2 = ops.tile([65, CH], F32, tag="O2", name=f"O2_{c}_{hp}")
    nkt = 4 * (c + 1)
    for kt in range(nkt):
        j = kt - 4 * c
        qoff = 128 * j if j > 0 else 0
        ksl = bass.ts(kt, 128)
        ST12 = stps.tile([128, 2, CH], F32, tag="ST", name=f"ST_{c}_{hp}_{kt}")
        for idx in (0, 1):
            bp = idx * 64
            nc.tensor.matmul(
                ST12[:, idx, qoff:CH],
                KT[hp][bp : bp + 64, ksl],
                QT[hp][bp : bp + 64, bass.ds(c * CH + qoff, CH - qoff)],
                start=True, stop=True,
                tile_position=(bp, 0),
            )
        E12 = epool.tile([128, 2, CH], BF16, tag="E", name=f"E_{c}_{hp}_{kt}")
        nc.scalar.activation(
            E12[:, :, qoff:CH], ST12[:, :, qoff:CH], EXPF, scale=0.125
        )
        if j >= 0:
            dsl = bass.ds(qoff, 128)
            nc.vector.tensor_mul(E12[:, 0, dsl], E12[:, 0, dsl], mask_sb)
            nc.vector.tensor_mul(E12[:, 1, dsl], E12[:, 1, dsl], mask_sb)
        for idx, Ox in ((0, O1), (1, O2)):
            h = 2 * hp + idx
            nc.tensor.matmul(
                Ox[:, qoff:CH],
                VA[kt][:, h * 65 : (h + 1) * 65],
                E12[:, idx, qoff:CH],
                start=(kt == 0), stop=(kt == nkt - 1),
            )
    # evict: copy O to SBUF (frees PSUM banks), wide-lane reciprocal of the
    # softmax sums via a [32,32] reshape, DRAM stride-0 broadcast, normalize.
    # (An outer-product PSUM broadcast was tried instead of the DRAM
    # round-trip: it stalls the strict-FIFO PE queue on the DVE reciprocal
    # chain at every head-pair transition and re-triggers HAM throttling.)
    hslot = c * 4 + hp
    Ss = []
    g = gpool.tile([32, 32], F32, tag="g", name=f"g{hslot}")
    for idx, Ox in ((0, O1), (1, O2)):
        S = spool.tile([65, CH], F32, tag="S", name=f"S{hslot}_{idx}")
        nc.vector.tensor_copy(S, Ox)
        nc.gpsimd.dma_start(out=g[16 * idx : 16 * idx + 16, :], in_=S[64:65, :])
        Ss.append(S)
    gr = gpool.tile([32, 32], F32, tag="gr", name=f"gr{hslot}")
    nc.vector.reciprocal(gr, g)
    nc.gpsimd.dma_start(out=rb[hslot : hslot + 1, :], in_=gr)
    src = rb[hslot : hslot + 1, :]
    bc1 = bcpool.tile([64, CH], F32, tag="bc1", name=f"bc1_{hslot}")
    nc.gpsimd.dma_start(
        out=bc1,
        in_=bass.AP(tensor=src.tensor, offset=src.offset, ap=[[0, 64], [1, CH]]),
    )
    bc2 = bcpool.tile([64, CH], F32, tag="bc2", name=f"bc2_{hslot}")
    nc.gpsimd.dma_start(
        out=bc2,
        in_=bass.AP(tensor=src.tensor, offset=src.offset + CH, ap=[[0, 64], [1, CH]]),
    )
    nc.vector.tensor_mul(OT[hp][0:64, csl], Ss[0][0:64, :], bc1)
    tmp = tpool.tile([64, CH], BF16, tag="tmp", name=f"tmp{hslot}")
    nc.vector.tensor_mul(tmp, Ss[1][0:64, :], bc2)
    nc.gpsimd.dma_start(out=OT[hp][64:128, csl], in_=tmp)


def _body(tc):
    nc = tc.nc
    dram = {t.name: t for t in _CACHE["dram"]}
    out = dram["out"]
    rb = _CACHE["rb"]

    # ---- persistent tiles (one bufs=1 pool, distinct tags -> own slots) ----
    import contextlib
    _pc = contextlib.ExitStack()
    persist = _pc.enter_context(tc.tile_pool(name="persist", bufs=1))

    def P(shape, dt, name):
        return persist.tile(shape, dt, tag=name, name=name)

    wq_sb = P([128, 8, M], BF16, "wq_sb")
    wk_sb = P([128, 8, M], BF16, "wk_sb")
    wv_sb = P([128, 8, M], BF16, "wv_sb")
    dw_sb = P([128, 4, D], BF16, "dw_sb")
    bqT_sb = P([128, 4], F32, "bqT_sb")
    bkT_sb = P([128, 4], F32, "bkT_sb")
    bvb_sb = P([128, M], F32, "bvb_sb")
    mask_sb = P([128, 128], BF16, "mask_sb")

    wqr = dram["wqT"][:, :].rearrange("(t p) m -> p t m", p=128)
    wkr = dram["wkT"][:, :].rearrange("(t p) m -> p t m", p=128)
    wvr = dram["wvT"][:, :].rearrange("(t p) m -> p t m", p=128)

    def wdma(i):
        # bulk weight loads, interleaved with chunk-0 input DMAs
        if i == 0:
            nc.sync.dma_start(out=wq_sb, in_=wqr)
        elif i == 1:
            nc.sync.dma_start(out=wk_sb, in_=wkr)
        else:
            nc.sync.dma_start(out=wv_sb, in_=wvr)

    QT = {mt: P([128, S], BF16, f"QT{mt}") for mt in range(4)}
    KT = {mt: P([128, S], BF16, f"KT{mt}") for mt in range(4)}
    VA = {st: P([128, HPC * 65], BF16, f"VA{st}") for st in range(NKT)}
    OT = {hp: P([128, S], BF16, f"OT{hp}") for hp in range(4)}
    for st in range(NKT):
        nc.vector.memset(VA[st], 1.0)

    qTr = dram["qT"][:, :].rearrange("(t p) s -> p t s", p=128)
    kTr = dram["kT"][:, :].rearrange("(t p) s -> p t s", p=128)
    vTr = dram["vT"][:, :].rearrange("(t p) s -> p t s", p=128)
    ptens = (qTr, kTr, vTr, wq_sb, wk_sb, wv_sb, bqT_sb, bkT_sb, bvb_sb, QT, KT, VA)
    atens = (QT, KT, VA, OT, mask_sb, rb)

    with (
        tc.tile_pool(name="xch", bufs=2) as xch,
        tc.tile_pool(name="stps", bufs=2, space="PSUM") as stps,
        tc.tile_pool(name="ops", bufs=1, space="PSUM") as ops,
        tc.tile_pool(name="pjps", bufs=2, space="PSUM") as pjps,
        tc.tile_pool(name="epool", bufs=3) as epool,
        tc.tile_pool(name="spool", bufs=3) as spool,
        tc.tile_pool(name="gpool", bufs=2) as gpool,
        tc.tile_pool(name="bcpool", bufs=3) as bcpool,
        tc.tile_pool(name="tpool", bufs=2) as tpool,
        tc.tile_pool(name="osb", bufs=3) as osb,
    ):
        apools = (stps, ops, epool, spool, gpool, bcpool, tpool)

        def dense_st(st):
            ssl = bass.ts(st, 128)
            o_sb = osb.tile([128, D], BF16, tag="o_sb", name=f"o_sb{st}")
            for nh in range(2):
                ps = pjps.tile([128, 512], F32, tag="pj", name=f"dn{st}_{nh}")
                for mt in range(4):
                    nc.tensor.matmul(
                        ps, OT[mt][:, ssl], dw_sb[:, mt, bass.ts(nh, 512)],
                        start=(mt == 0), stop=(mt == 3),
                    )
                nc.vector.tensor_copy(o_sb[:, bass.ts(nh, 512)], ps)
            nc.sync.dma_start(out=out[bass.ts(st, 128), :], in_=o_sb)

        # emission order = Tile priority. Attention chunk c only needs
        # QT cols [512c:512c+512) and KT/VA cols [0:512c+512) — i.e. proj
        # chunks 0..c — so each attention chunk is emitted right after the
        # proj chunk it unblocks. Later proj chunks and per-chunk dense
        # matmuls then fill attention's ACT-bound PE gaps.
        _proj_chunk(tc, 0, xch, pjps, ptens, wdma=wdma)
        nc.sync.dma_start(out=bqT_sb, in_=dram["bqT"][:, :])
        nc.sync.dma_start(out=bkT_sb, in_=dram["bkT"][:, :])
        nc.sync.dma_start(out=mask_sb, in_=dram["mask"][:, :])
        nc.sync.dma_start(out=bvb_sb, in_=dram["bvb"][:, :])
        for c in range(NCH):
            for hp in range(4):
                _attn_chunk(tc, c, hp, apools, atens)
            if c == 0:
                nc.sync.dma_start(
                    out=dw_sb,
                    in_=dram["dwT"][:, :].rearrange("(t p) d -> p t d", p=128),
                )
            if c + 1 < NCH:
                _proj_chunk(tc, c + 1, xch, pjps, ptens)
            if c > 0:
                for st in range(4 * (c - 1), 4 * c):
                    dense_st(st)
        for st in range(4 * (NCH - 1), 4 * NCH):
            dense_st(st)
    _pc.close()


def _legalize_dma_waits(nc):
    """Walrus accepts only one sync wait per instruction (EventSemaphore: 2,
    Drain: special-cased). Spill extra waits onto preceding InstEventSemaphore
    ops on the same engine sequencer."""
    for f in nc.m.functions:
        for blk in f.blocks:
            new_insts = []
            for inst in blk.instructions:
                si = getattr(inst, "sync_info", None)
                exempt = isinstance(inst, mybir.InstEventSemaphore)
                if not exempt and si is not None and len(si.on_wait) > 1:
                    waits = list(si.on_wait)
                    extra, keep = waits[:-1], waits[-1:]
                    while extra:
                        chunk, extra = extra[:2], extra[2:]
                        new_insts.append(
                            mybir.InstEventSemaphore(
                                name=nc.get_next_instruction_name(),
                                engine=inst.engine,
                                ins=[],
                                outs=[],
                                sync_info=mybir.SyncInfo(on_wait=chunk, on_update=[]),
                            )
                        )
                    inst.sync_info = mybir.SyncInfo(
                        on_wait=keep, on_update=list(si.on_update)
                    )
                new_insts.append(inst)
            blk.instructions[:] = new_insts


def _build():
    nc = bass.Bass()
    dram = [
        nc.declare_dram_parameter("qT", [D, S], BF16, isOutput=False),
        nc.declare_dram_parameter("kT", [D, S], BF16, isOutput=False),
        nc.declare_dram_parameter("vT", [D, S], BF16, isOutput=False),
        nc.declare_dram_parameter("wqT", [D, M], BF16, isOutput=False),
        nc.declare_dram_parameter("wkT", [D, M], BF16, isOutput=False),
        nc.declare_dram_parameter("wvT", [D, M], BF16, isOutput=False),
        nc.declare_dram_parameter("dwT", [M, D], BF16, isOutput=False),
        nc.declare_dram_parameter("bqT", [128, 4], F32, isOutput=False),
        nc.declare_dram_parameter("bkT", [128, 4], F32, isOutput=False),
        nc.declare_dram_parameter("bvb", [128, M], F32, isOutput=False),
        nc.declare_dram_parameter("mask", [128, 128], BF16, isOutput=False),
        nc.declare_dram_parameter("out", [S, D], BF16, isOutput=True),
    ]
    _CACHE["dram"] = dram
    _CACHE["rb"] = nc.dram_tensor("rb", [16, 2 * CH], F32)
    with tile.TileContext(nc) as tc:
        _body(tc)
    _legalize_dma_waits(nc)
    return nc


def _get_nc():
    if "nc" not in _CACHE:
        _CACHE["nc"] = _build()
    return _CACHE["nc"]


def _make_in_maps(q, k, v, wq_w, wq_b, wk_w, wk_b, wv_w, wv_b, dense_w, dense_b):
    q, k, v = (np.asarray(x, np.float32) for x in (q, k, v))
    mask = np.triu(np.ones((128, 128), np.float32)).astype(bf16)
    in_maps = []
    for core in range(NCORES):
        b, g = divmod(core, 2)
        hs = slice(g * M, (g + 1) * M)
        in_maps.append(
            {
                "qT": q[b].T.astype(bf16),
                "kT": k[b].T.astype(bf16),
                "vT": v[b].T.astype(bf16),
                "wqT": np.asarray(wq_w)[hs].T.astype(bf16),
                "wkT": np.asarray(wk_w)[hs].T.astype(bf16),
                "wvT": np.asarray(wv_w)[hs].T.astype(bf16),
                "dwT": np.asarray(dense_w)[:, hs].T.astype(bf16),
                "bqT": np.ascontiguousarray(
                    np.asarray(wq_b)[hs].reshape(4, 128).T
                ).astype(np.float32),
                "bkT": np.ascontiguousarray(
                    np.asarray(wk_b)[hs].reshape(4, 128).T
                ).astype(np.float32),
                "bvb": np.ascontiguousarray(
                    np.broadcast_to(np.asarray(wv_b)[hs], (128, M))
                ).astype(np.float32),
                "mask": mask,
            }
        )
    return in_maps


def kernel(q, k, v, wq_w, wq_b, wk_w, wk_b, wv_w, wv_b, dense_w, dense_b):
    nc = _get_nc()
    in_maps = _make_in_maps(
        q, k, v, wq_w, wq_b, wk_w, wk_b, wv_w, wv_b, dense_w, dense_b
    )
    res = run_bass_kernel_spmd(nc, in_maps, list(range(NCORES)))
    _CACHE["last_res"] = res
    outs = [r["out"] for r in res.results]
    final = np.empty((B, S, D), np.float32)
    db = np.asarray(dense_b, np.float32)
    for b in range(B):
        final[b] = (
            outs[2 * b].astype(np.float32)
            + outs[2 * b + 1].astype(np.float32)
            + db[None, :]
        )
    return final
